# revision 1
# baseline (speedup 1.0000x reference)
"""Multi-head self-attention (B=4, T=2048, D=1024, H=16) on 8 TRN2 NeuronCores.

Sharding: tensor-parallel over heads. Core c owns heads (2c, 2c+1):
  - W_Q/W_K/W_V rows [128c, 128c+128) -> per-core q/k/v of shape [T*B, 128]
  - attention for its 2 heads (causal, block-skipped)
  - partial output projection through W_O columns [128c, 128c+128)
Host sums the 8 partial outputs (the row-parallel W_O reduction).

Layouts (on device, per core):
  xT   [8, 128, 8192]  : x^T tiled over model dim (bf16)
  qT/kT per batch [128, 2048] : transposed q/k (partition = head dim, 2 heads)
  vaug per batch [128, 16, 130] : v token-major, per head [64 dims | ones col]
  scores^T tiles [128 k-tok, 2*512 q] so the AV matmul contracts k on
  partitions; softmax denominator = ones-column row of the AV output;
  1/denom = exp(-ln(denom)) on ACT (ln+exp share one table set).
"""

import os
import sys

import numpy as np

if "/opt/trn_rl_repo" not in sys.path:
    sys.path.insert(0, "/opt/trn_rl_repo")

import ml_dtypes

B, T, D, NH, DH = 4, 2048, 1024, 16, 64
NT = B * T          # 8192 tokens
MT = D // 128       # 8 model-dim tiles
NCH = NT // 512     # 16 token chunks
N_CORES = 8

_cache = {}


def _build_nc():
    from contextlib import ExitStack

    import concourse.mybir as mybir
    import concourse.tile as tile
    from concourse import bacc

    BF = mybir.dt.bfloat16
    F32 = mybir.dt.float32
    EXP = mybir.ActivationFunctionType.Exp
    LN = mybir.ActivationFunctionType.Ln

    nc = bacc.Bacc("TRN2", target_bir_lowering=False, debug=False)

    xT_d = nc.dram_tensor("xT", [MT, 128, NT], BF, kind="ExternalInput")
    wq_d = nc.dram_tensor("wqT", [MT, 128, 128], BF, kind="ExternalInput")
    wk_d = nc.dram_tensor("wkT", [MT, 128, 128], BF, kind="ExternalInput")
    wv_d = nc.dram_tensor("wvT", [MT, 128, 128], BF, kind="ExternalInput")
    wo_d = nc.dram_tensor("woT", [128, D], BF, kind="ExternalInput")
    cm_d = nc.dram_tensor("cmask", [4, 128, 512], F32, kind="ExternalInput")
    out_d = nc.dram_tensor("out", [NT, D], F32, kind="ExternalOutput")

    with tile.TileContext(nc) as tc, ExitStack() as ctx:
        pers = ctx.enter_context(tc.tile_pool(name="pers", bufs=1))
        qTb = [pers.tile([128, T], BF, tag=f"qT{b}", name=f"qT{b}") for b in range(B)]
        kTb = [pers.tile([128, T], BF, tag=f"kT{b}", name=f"kT{b}") for b in range(B)]
        vb = [pers.tile([128, 16, 130], BF, tag=f"v{b}", name=f"v{b}") for b in range(B)]
        masks = pers.tile([128, 4, 512], F32)
        wq = pers.tile([128, MT, 128], BF)
        wk = pers.tile([128, MT, 128], BF)
        wv = pers.tile([128, MT, 128], BF)
        wo = pers.tile([128, D], BF)
        ones64 = pers.tile([1, 64], BF)

        nc.vector.memset(ones64, 1.0)
        for b in range(B):
            nc.vector.memset(vb[b], 1.0)
        for i in range(4):
            nc.sync.dma_start(out=masks[:, i, :], in_=cm_d[i])
        for mt in range(MT):
            nc.sync.dma_start(out=wq[:, mt, :], in_=wq_d[mt])
            nc.sync.dma_start(out=wk[:, mt, :], in_=wk_d[mt])
            nc.sync.dma_start(out=wv[:, mt, :], in_=wv_d[mt])
        nc.sync.dma_start(out=wo[:], in_=wo_d[:])

        # ---- Phase 1: q/k/v projections, one 512-token chunk at a time ----
        with tc.tile_pool(name="xc", bufs=3) as xpool, \
             tc.tile_pool(name="pq", bufs=2, space="PSUM") as pq_pool, \
             tc.tile_pool(name="pk", bufs=2, space="PSUM") as pk_pool, \
             tc.tile_pool(name="pv", bufs=2, space="PSUM") as pv_pool:
            for c in range(NCH):
                b, lc = c // 4, c % 4
                cs = slice(lc * 512, (lc + 1) * 512)
                xc = xpool.tile([128, MT, 512], BF)
                for mt in range(MT):
                    nc.sync.dma_start(out=xc[:, mt, :],
                                      in_=xT_d[mt, :, c * 512:(c + 1) * 512])
                pq = pq_pool.tile([128, 512], F32)
                pk = pk_pool.tile([128, 512], F32)
                pv = pv_pool.tile([128, 4, 128], F32)
                for mt in range(MT):
                    nc.tensor.matmul(pq, wq[:, mt, :], xc[:, mt, :],
                                     start=(mt == 0), stop=(mt == MT - 1))
                for mt in range(MT):
                    nc.tensor.matmul(pk, wk[:, mt, :], xc[:, mt, :],
                                     start=(mt == 0), stop=(mt == MT - 1))
                for tt in range(4):
                    for mt in range(MT):
                        nc.tensor.matmul(pv[:, tt, :],
                                         xc[:, mt, tt * 128:(tt + 1) * 128],
                                         wv[:, mt, :],
                                         start=(mt == 0), stop=(mt == MT - 1))
                nc.vector.tensor_copy(out=qTb[b][:, cs], in_=pq)
                nc.vector.tensor_copy(out=kTb[b][:, cs], in_=pk)
                for h in range(2):
                    nc.vector.tensor_copy(
                        out=vb[b][:, lc * 4:(lc + 1) * 4, 65 * h:65 * h + 64],
                        in_=pv[:, :, 64 * h:64 * h + 64])

        # ---- Phase 2: causal attention + partial output projection ----
        # scores^T computed in k-supertiles of 2x128 so exp amortizes the
        # ACT fixed overhead over 1024 columns.
        with tc.tile_pool(name="ps_s", bufs=2, space="PSUM") as sp, \
             tc.tile_pool(name="ps_av", bufs=1, space="PSUM") as avp, \
             tc.tile_pool(name="ps_bc", bufs=1, space="PSUM") as bcp, \
             tc.tile_pool(name="ps_o", bufs=1, space="PSUM") as op_, \
             tc.tile_pool(name="esb", bufs=4) as ep, \
             tc.tile_pool(name="nrm", bufs=4) as nrm, \
             tc.tile_pool(name="osb", bufs=3) as osb, \
             tc.tile_pool(name="hop", bufs=2) as hop:
            for b in range(B):
                hoT = hop.tile([128, T], BF)
                for qc in range(4):
                    qs = slice(qc * 512, (qc + 1) * 512)
                    pavs = [avp.tile([65, 512], F32, tag=f"pav{h}", name=f"pav{h}")
                            for h in range(2)]
                    ns = 2 * (qc + 1)      # k-supertiles of 256 tokens
                    for s in range(ns):
                        for h in range(2):
                            hp = 64 * h
                            pss = sp.tile([128, 2, 512], F32)
                            for j in range(2):
                                kt = 2 * s + j
                                ko = kt * 128
                                nc.tensor.matmul(
                                    pss[:, j, :],
                                    kTb[b][hp:hp + 64, ko:ko + 128],
                                    qTb[b][hp:hp + 64, qs],
                                    start=True, stop=True)
                            if s >= 2 * qc:
                                j0 = 2 * (s - 2 * qc)
                                nc.vector.tensor_add(
                                    pss[:], pss[:], masks[:, j0:j0 + 2, :])
                            ex = ep.tile([128, 2, 512], BF)
                            nc.scalar.activation(out=ex[:], in_=pss[:],
                                                 func=EXP, scale=0.125)
                            for j in range(2):
                                kt = 2 * s + j
                                nc.tensor.matmul(
                                    pavs[h],
                                    vb[b][:, kt, 65 * h:65 * h + 65],
                                    ex[:, j, :],
                                    start=(s == 0 and j == 0),
                                    stop=(s == ns - 1 and j == 1))
                    for h in range(2):
                        hp = 64 * h
                        lnd = nrm.tile([1, 512], F32)
                        nc.scalar.activation(out=lnd, in_=pavs[h][64:65, :],
                                             func=LN)
                        inv = nrm.tile([1, 512], BF)
                        with nc.allow_low_precision(
                                reason="softmax 1/denom via exp(-ln d)"):
                            nc.scalar.activation(out=inv, in_=lnd, func=EXP,
                                                 scale=-1.0)
                        pbc = bcp.tile([64, 512], F32)
                        nc.tensor.matmul(pbc, ones64[:], inv[:],
                                         start=True, stop=True)
                        invb = nrm.tile([64, 512], F32)
                        nc.vector.tensor_copy(out=invb, in_=pbc)
                        nc.vector.tensor_mul(hoT[hp:hp + 64, qs],
                                             pavs[h][0:64, :], invb)
                for tt in range(T // 128):
                    to = b * T + tt * 128
                    for oc in range(2):
                        po = op_.tile([128, 512], F32)
                        nc.tensor.matmul(po, hoT[:, tt * 128:(tt + 1) * 128],
                                         wo[:, oc * 512:(oc + 1) * 512],
                                         start=True, stop=True)
                        ost = osb.tile([128, 512], F32)
                        nc.vector.tensor_copy(out=ost, in_=po)
                        nc.sync.dma_start(
                            out=out_d[to:to + 128, oc * 512:(oc + 1) * 512],
                            in_=ost)
    nc.compile()
    return nc


def _get_nc():
    if "nc" not in _cache:
        _cache["nc"] = _build_nc()
    return _cache["nc"]


def _bf(a):
    return np.ascontiguousarray(a, dtype=np.float32).astype(ml_dtypes.bfloat16)


def make_in_maps(x, W_Q, W_K, W_V, W_O):
    xT = _bf(x.reshape(NT, D).T).reshape(MT, 128, NT)
    cmask = np.full((4, 128, 512), -1e10, dtype=np.float32)
    for t in range(4):
        for kp in range(128):
            cmask[t, kp, t * 128 + kp:] = 0.0
    in_maps = []
    for c in range(N_CORES):
        rs = slice(c * 128, (c + 1) * 128)
        in_maps.append({
            "xT": xT,
            "wqT": _bf(W_Q[rs, :].T).reshape(MT, 128, 128),
            "wkT": _bf(W_K[rs, :].T).reshape(MT, 128, 128),
            "wvT": _bf(W_V[rs, :].T).reshape(MT, 128, 128),
            "woT": _bf(W_O[:, rs].T),
            "cmask": cmask,
        })
    return in_maps


def _ensure_ntff_hook():
    """Install antenv.axon_hooks shim (missing in this image) so
    run_bass_kernel_spmd(trace=True) can capture NTFF profiles."""
    try:
        from antenv import axon_hooks  # noqa: F401
        return True
    except ImportError:
        pass
    try:
        import contextlib
        import ctypes
        import types

        import antenv

        so_path = "/opt/axon/libaxon_pjrt.so"
        lib = ctypes.CDLL(so_path)
        if not hasattr(lib, "axon_start_nrt_profile"):
            return False
        lib.axon_start_nrt_profile.argtypes = [
            ctypes.POINTER(ctypes.c_int64), ctypes.c_size_t]
        lib.axon_start_nrt_profile.restype = ctypes.c_int64
        lib.axon_stop_nrt_profile.argtypes = [ctypes.c_char_p]
        lib.axon_stop_nrt_profile.restype = ctypes.c_int64

        @contextlib.contextmanager
        def _hook(output_dir, device_ids):
            import jax
            jax.devices()
            if device_ids:
                ids = (ctypes.c_int64 * len(device_ids))(*device_ids)
                rc = lib.axon_start_nrt_profile(ids, len(device_ids))
            else:
                rc = lib.axon_start_nrt_profile(None, 0)
            if rc != 0:
                raise RuntimeError(f"axon_start_nrt_profile rc={rc}")
            try:
                yield
            finally:
                n = lib.axon_stop_nrt_profile(str(output_dir).encode())
                print(f"ntff profile: {n} file(s) -> {output_dir}",
                      file=sys.stderr)

        mod = types.ModuleType("antenv.axon_hooks")
        mod.get_axon_ntff_profile_hook = lambda: _hook
        mod.set_axon_ntff_profile_hook = lambda h: None
        sys.modules["antenv.axon_hooks"] = mod
        antenv.axon_hooks = mod
        return True
    except Exception as e:  # pragma: no cover
        print(f"ntff hook install failed: {e}", file=sys.stderr)
        return False


def bench_pjrt(in_maps, n_iters=8):
    """Run the SPMD program with device-resident inputs; return (results,
    per-iter wall times). Mirrors bass2jax.run_bass_via_pjrt but reuses the
    jitted executable and chains donated output buffers for timing."""
    import time

    import jax
    import concourse.mybir as mybir
    from jax.sharding import Mesh, NamedSharding, PartitionSpec
    from jax.experimental.shard_map import shard_map
    from concourse import bass2jax

    nc = _get_nc()
    bass2jax.install_neuronx_cc_hook()

    part_name = nc.partition_id_tensor.name if nc.partition_id_tensor else None
    in_names, out_names, out_avals, zero_outs = [], [], [], []
    for alloc in nc.m.functions[0].allocations:
        if not isinstance(alloc, mybir.MemoryLocationSet):
            continue
        name = alloc.memorylocations[0].name
        if alloc.kind == "ExternalInput":
            if name != part_name:
                in_names.append(name)
        elif alloc.kind == "ExternalOutput":
            shape = tuple(alloc.tensor_shape)
            dtype = mybir.dt.np(alloc.dtype)
            out_names.append(name)
            out_avals.append(jax.core.ShapedArray(shape, dtype))
            zero_outs.append(np.zeros(shape, dtype))
    n_params = len(in_names)
    all_names = in_names + out_names
    if part_name is not None:
        all_names = all_names + [part_name]

    def _body(*args):
        operands = list(args)
        if part_name is not None:
            operands.append(bass2jax.partition_id_tensor())
        outs = bass2jax._bass_exec_p.bind(
            *operands,
            out_avals=tuple(out_avals),
            in_names=tuple(all_names),
            out_names=tuple(out_names),
            lowering_input_output_aliases=(),
            sim_require_finite=True,
            sim_require_nnan=True,
            nc=nc,
        )
        return tuple(outs)

    n_cores = len(in_maps)
    devices = jax.devices()[:n_cores]
    mesh = Mesh(np.asarray(devices), ("core",))
    donate = tuple(range(n_params, n_params + len(out_names)))
    sharded = jax.jit(
        shard_map(_body, mesh=mesh,
                  in_specs=(PartitionSpec("core"),) * (n_params + len(out_names)),
                  out_specs=(PartitionSpec("core"),) * len(out_names),
                  check_rep=False),
        donate_argnums=donate, keep_unused=True)

    concat_in = [
        np.concatenate([np.asarray(in_maps[c][k]) for c in range(n_cores)],
                       axis=0) for k in in_names]
    concat_zeros = [np.zeros((n_cores * z.shape[0], *z.shape[1:]), z.dtype)
                    for z in zero_outs]
    sh = NamedSharding(mesh, PartitionSpec("core"))
    dev_in = [jax.device_put(a, sh) for a in concat_in]
    outs = sharded(*dev_in, *[jax.device_put(z, sh) for z in concat_zeros])
    jax.block_until_ready(outs)
    first = [np.asarray(o) for o in outs]

    times = []
    for _ in range(n_iters):
        t0 = time.perf_counter()
        outs = sharded(*dev_in, *outs)
        jax.block_until_ready(outs)
        times.append(time.perf_counter() - t0)

    results = [
        {name: first[i].reshape(n_cores, *out_avals[i].shape)[c]
         for i, name in enumerate(out_names)}
        for c in range(n_cores)
    ]
    return results, times


def kernel(x, W_Q, W_K, W_V, W_O):
    import concourse.bass_utils as bass_utils

    x = np.asarray(x, dtype=np.float32)
    in_maps = make_in_maps(x, np.asarray(W_Q, np.float32),
                           np.asarray(W_K, np.float32),
                           np.asarray(W_V, np.float32),
                           np.asarray(W_O, np.float32))
    nc = _get_nc()
    trace = bool(int(os.environ.get("MHSA_TRACE", "0")))
    tmpdir = None
    if trace:
        trace = _ensure_ntff_hook()
    if trace:
        import tempfile
        tmpdir = tempfile.mkdtemp(prefix="mhsa_ntff_")
        _cache["trace_dir"] = tmpdir
        # no cloud creds in this container; keep artifacts local
        bass_utils.upload_artifacts = lambda d: f"local://{d}"
    res = bass_utils.run_bass_kernel_spmd(
        nc, in_maps, list(range(N_CORES)), trace=trace, tmpdir=tmpdir)
    _cache["last_results"] = res
    out = np.zeros((NT, D), dtype=np.float32)
    for r in res.results:
        out += np.asarray(r["out"], dtype=np.float32)
    return out.reshape(B, T, D)



# revision 6
# speedup vs baseline: 155.3865x; 155.3865x over previous
"""Multi-head self-attention (B=4, T=2048, D=1024, H=16) on 8 TRN2 NeuronCores.

Sharding: tensor-parallel over heads. Core c owns heads (2c, 2c+1):
  - W_Q/W_K/W_V rows [128c, 128c+128) -> per-core q/k/v of shape [T*B, 128]
  - causal attention for its 2 heads
  - partial output projection through W_O columns [128c, 128c+128)
Host sums the 8 partial outputs (the row-parallel W_O reduction).

Per-core schedule (batch-fused so engines stay dense):
  for b in 0..3:
    phase1(b): q/k/v projections for batch b, one 512-token chunk at a
      time (x streamed chunk-major, ONE dma per chunk).
    attn(b), per 512-q chunk: per 128-k-tile: h-packed score MMs (heads
      at array rows 0-63/64-127, concurrent) -> diag-mask add (DVE) ->
      exp (ACT, bf16 out) -> AV MMs accumulating [64 v-dims | ones] so
      row 64 is the softmax denominator. Diagonal k-tiles are column-
      sliced so fully-masked blocks are never computed.
      Chunk tail: 1/denom via DVE reciprocal_approx_fast (no ACT table
      swaps), GpSimd partition_broadcast, DVE mul into hoT, then the
      output projection + DMA for that chunk's 4 token-tiles.
  The dataflow scheduler overlaps attn(b) with phase1(b+1), keeping the
  PE dense (HAM stays at K=8/8) while ACT runs the exp stream.
"""

import os
import sys

import numpy as np

if "/opt/trn_rl_repo" not in sys.path:
    sys.path.insert(0, "/opt/trn_rl_repo")

import ml_dtypes

B, T, D, NH, DH = 4, 2048, 1024, 16, 64
NT = B * T          # 8192 tokens
MT = D // 128       # 8 model-dim tiles
NCH = NT // 512     # 16 token chunks
N_CORES = 8

_cache = {}


def _build_nc():
    from contextlib import ExitStack

    import concourse.mybir as mybir
    import concourse.tile as tile
    from concourse import bacc

    BF = mybir.dt.bfloat16
    F32 = mybir.dt.float32
    EXP = mybir.ActivationFunctionType.Exp
    LN = mybir.ActivationFunctionType.Ln

    nc = bacc.Bacc("TRN2", target_bir_lowering=False, debug=False)

    # x chunk-major: [chunk, partition, mt, col]
    xT_d = nc.dram_tensor("xT", [NCH, 128, MT, 512], BF, kind="ExternalInput")
    wq_d = nc.dram_tensor("wqT", [MT, 128, 128], BF, kind="ExternalInput")
    wk_d = nc.dram_tensor("wkT", [MT, 128, 128], BF, kind="ExternalInput")
    wv_d = nc.dram_tensor("wvT", [MT, 128, 128], BF, kind="ExternalInput")
    wo_d = nc.dram_tensor("woT", [128, D], BF, kind="ExternalInput")
    cm_d = nc.dram_tensor("cmask", [128, 2, 128], F32, kind="ExternalInput")
    out_d = nc.dram_tensor("out", [NT, D], F32, kind="ExternalOutput")

    with tile.TileContext(nc) as tc, ExitStack() as ctx:
        pers = ctx.enter_context(tc.tile_pool(name="pers", bufs=1))
        wq = pers.tile([128, MT, 128], BF)
        wk = pers.tile([128, MT, 128], BF)
        wv = pers.tile([128, MT, 128], BF)
        wo = pers.tile([128, D], BF)
        cmask = pers.tile([128, 2, 128], F32)

        with tc.tile_pool(name="xc", bufs=3) as xpool, \
             tc.tile_pool(name="qt", bufs=2) as qtp, \
             tc.tile_pool(name="kt", bufs=2) as ktp, \
             tc.tile_pool(name="vbp", bufs=2) as vbp, \
             tc.tile_pool(name="hop", bufs=2) as hop, \
             tc.tile_pool(name="exp", bufs=4) as exp_pool, \
             tc.tile_pool(name="uhp", bufs=2) as uhp, \
             tc.tile_pool(name="lnp", bufs=2) as lnp, \
             tc.tile_pool(name="inv", bufs=2) as invp, \
             tc.tile_pool(name="osb", bufs=3) as osbp, \
             tc.tile_pool(name="p1", bufs=2, space="PSUM") as p1p, \
             tc.tile_pool(name="sp", bufs=2, space="PSUM") as spp, \
             tc.tile_pool(name="avp", bufs=1, space="PSUM") as avp:

            first = True
            for b in range(B):
                # ---- phase 1: q/k/v projections for batch b ----
                qt = qtp.tile([128, T], BF, tag="qt", name="qt")
                kt = ktp.tile([128, T], BF, tag="kt", name="kt")
                vb = vbp.tile([128, 16, 130], BF, tag="vb", name="vb")
                for lc in range(4):
                    c = 4 * b + lc
                    cs = slice(lc * 512, (lc + 1) * 512)
                    xc = xpool.tile([128, MT, 512], BF, tag="xc", name="xc")
                    nc.sync.dma_start(out=xc, in_=xT_d[c])
                    if first:
                        # weights after the first x chunk so PE starts asap
                        for mt in range(MT):
                            nc.sync.dma_start(out=wq[:, mt, :], in_=wq_d[mt])
                            nc.sync.dma_start(out=wk[:, mt, :], in_=wk_d[mt])
                            nc.sync.dma_start(out=wv[:, mt, :], in_=wv_d[mt])
                        nc.sync.dma_start(out=wo[:], in_=wo_d[:])
                        nc.sync.dma_start(out=cmask, in_=cm_d[:])
                        first = False
                    pq = p1p.tile([128, 512], F32, tag="p1", name="pq")
                    for mt in range(MT):
                        nc.tensor.matmul(pq, wq[:, mt, :], xc[:, mt, :],
                                         start=(mt == 0), stop=(mt == MT - 1))
                    nc.vector.tensor_copy(out=qt[:, cs], in_=pq)
                    pk = p1p.tile([128, 512], F32, tag="p1", name="pk")
                    for mt in range(MT):
                        nc.tensor.matmul(pk, wk[:, mt, :], xc[:, mt, :],
                                         start=(mt == 0), stop=(mt == MT - 1))
                    nc.vector.tensor_copy(out=kt[:, cs], in_=pk)
                    pv = p1p.tile([128, 4, 128], F32, tag="p1", name="pv")
                    for tt in range(4):
                        for mt in range(MT):
                            nc.tensor.matmul(pv[:, tt, :],
                                             xc[:, mt, tt * 128:(tt + 1) * 128],
                                             wv[:, mt, :],
                                             start=(mt == 0), stop=(mt == MT - 1))
                    for h in range(2):
                        nc.vector.tensor_copy(
                            out=vb[:, lc * 4:(lc + 1) * 4, 65 * h:65 * h + 64],
                            in_=pv[:, :, 64 * h:64 * h + 64])
                # softmax-denominator ones columns
                nc.vector.memset(vb[:, :, 64:65], 1.0)
                nc.vector.memset(vb[:, :, 129:130], 1.0)

                # ---- phase 2: causal attention + projection for batch b ----
                hoT = hop.tile([128, T], BF, tag="hoT", name="hoT")
                for qc in range(4):
                    q0 = qc * 512
                    nk = 4 * qc + 4
                    pavs = [avp.tile([65, 512], F32, tag=f"pav{h}",
                                     name=f"pav{h}") for h in range(2)]
                    for kt_i in range(nk):
                        off = 128 * (kt_i - 4 * qc) if kt_i >= 4 * qc else 0
                        pss = spp.tile([128, 2, 512], F32, tag="pss", name="pss")
                        for h in range(2):
                            hp = 64 * h
                            nc.tensor.matmul(
                                pss[:, h, off:512],
                                kt[hp:hp + 64, kt_i * 128:(kt_i + 1) * 128],
                                qt[hp:hp + 64, q0 + off:q0 + 512],
                                start=True, stop=True)
                        if kt_i >= 4 * qc:
                            nc.vector.tensor_add(pss[:, :, off:off + 128],
                                                 pss[:, :, off:off + 128],
                                                 cmask)
                        ex = exp_pool.tile([128, 2, 512], BF, tag="ex", name="ex")
                        nc.scalar.activation(out=ex[:, :, off:512],
                                             in_=pss[:, :, off:512],
                                             func=EXP, scale=0.125)
                        for h in range(2):
                            nc.tensor.matmul(
                                pavs[h][:, off:512],
                                vb[:, kt_i, 65 * h:65 * h + 65],
                                ex[:, h, off:512],
                                start=(kt_i == 0), stop=(kt_i == nk - 1))
                    # chunk tail: stash unnormalized AV+denoms in SBUF
                    # (frees PSUM), 1/denom = exp(-ln d) on ACT (same
                    # table set as the score exps), broadcast, normalize.
                    uh = uhp.tile([128, 2, 512], BF, tag="uh", name="uh")
                    for h in range(2):
                        nc.vector.tensor_copy(out=uh[0:65, h, :],
                                              in_=pavs[h])
                    lnd = lnp.tile([1, 2, 512], F32, tag="lnd", name="lnd")
                    nc.scalar.activation(out=lnd, in_=uh[64:65, :, :],
                                         func=LN)
                    inv = lnp.tile([1, 2, 512], BF, tag="inv", name="inv")
                    with nc.allow_low_precision(
                            reason="softmax 1/denom via exp(-ln d)"):
                        nc.scalar.activation(out=inv, in_=lnd, func=EXP,
                                             scale=-1.0)
                    for h in range(2):
                        invb = invp.tile([64, 512], BF, tag=f"invb{h}",
                                         name=f"invb{h}")
                        nc.gpsimd.partition_broadcast(invb, inv[0:1, h, :])
                        nc.vector.tensor_mul(hoT[64 * h:64 * h + 64,
                                                 q0:q0 + 512],
                                             uh[0:64, h, :], invb)
                    # output projection for this chunk's 4 token-tiles
                    for tt in range(4 * qc, 4 * qc + 4):
                        osb = osbp.tile([128, D], F32, tag="osb", name="osb")
                        for oc in range(2):
                            po = p1p.tile([128, 512], F32, tag="p1", name="po")
                            nc.tensor.matmul(po,
                                             hoT[:, tt * 128:(tt + 1) * 128],
                                             wo[:, oc * 512:(oc + 1) * 512],
                                             start=True, stop=True)
                            nc.vector.tensor_copy(
                                out=osb[:, oc * 512:(oc + 1) * 512], in_=po)
                        to = b * T + tt * 128
                        nc.sync.dma_start(out=out_d[to:to + 128, :], in_=osb)
    # Compile with the activation-table chooser steered to the one set
    # that contains BOTH exp and ln, so the kernel needs a single
    # ACT_TABLE_LOAD (the default per-function choice alternates between
    # exp_and_others and natural_log, reloading tables at every switch).
    # Set ids stay valid: the dict keeps its size and insertion order,
    # only the membership used for selection is narrowed.
    import concourse.bacc as bacc_mod
    orig_tables = bacc_mod.get_activation_tables

    def _steered_tables(arch):
        tabs = orig_tables(arch)
        keep = "natural_log_exp_and_others"
        if keep in tabs:
            tabs = {name: (fns if name == keep else fns - tabs[keep])
                    for name, fns in tabs.items()}
        return tabs

    bacc_mod.get_activation_tables = _steered_tables
    try:
        nc.compile()
    finally:
        bacc_mod.get_activation_tables = orig_tables
    return nc


def _get_nc():
    if "nc" not in _cache:
        _cache["nc"] = _build_nc()
    return _cache["nc"]


def _bf(a):
    return np.ascontiguousarray(a, dtype=np.float32).astype(ml_dtypes.bfloat16)


def make_in_maps(x, W_Q, W_K, W_V, W_O):
    xT = _bf(x.reshape(NT, D).T)                      # [D, NT]
    # [chunk, partition, mt, col]
    xTc = np.ascontiguousarray(
        xT.reshape(MT, 128, NCH, 512).transpose(2, 1, 0, 3))
    cmask = np.zeros((128, 2, 128), dtype=np.float32)
    for kp in range(128):
        cmask[kp, :, :kp] = -1e10
    in_maps = []
    for c in range(N_CORES):
        rs = slice(c * 128, (c + 1) * 128)
        in_maps.append({
            "xT": xTc,
            "wqT": _bf(W_Q[rs, :].T).reshape(MT, 128, 128),
            "wkT": _bf(W_K[rs, :].T).reshape(MT, 128, 128),
            "wvT": _bf(W_V[rs, :].T).reshape(MT, 128, 128),
            "woT": _bf(W_O[:, rs].T),
            "cmask": cmask,
        })
    return in_maps


def _ensure_ntff_hook():
    """Install antenv.axon_hooks shim (missing in this image) so
    run_bass_kernel_spmd(trace=True) can capture NTFF profiles."""
    try:
        from antenv import axon_hooks  # noqa: F401
        return True
    except ImportError:
        pass
    try:
        import contextlib
        import ctypes
        import types

        import antenv

        so_path = "/opt/axon/libaxon_pjrt.so"
        lib = ctypes.CDLL(so_path)
        if not hasattr(lib, "axon_start_nrt_profile"):
            return False
        lib.axon_start_nrt_profile.argtypes = [
            ctypes.POINTER(ctypes.c_int64), ctypes.c_size_t]
        lib.axon_start_nrt_profile.restype = ctypes.c_int64
        lib.axon_stop_nrt_profile.argtypes = [ctypes.c_char_p]
        lib.axon_stop_nrt_profile.restype = ctypes.c_int64

        @contextlib.contextmanager
        def _hook(output_dir, device_ids):
            import jax
            jax.devices()
            if device_ids:
                ids = (ctypes.c_int64 * len(device_ids))(*device_ids)
                rc = lib.axon_start_nrt_profile(ids, len(device_ids))
            else:
                rc = lib.axon_start_nrt_profile(None, 0)
            if rc != 0:
                raise RuntimeError(f"axon_start_nrt_profile rc={rc}")
            try:
                yield
            finally:
                n = lib.axon_stop_nrt_profile(str(output_dir).encode())
                print(f"ntff profile: {n} file(s) -> {output_dir}",
                      file=sys.stderr)

        mod = types.ModuleType("antenv.axon_hooks")
        mod.get_axon_ntff_profile_hook = lambda: _hook
        mod.set_axon_ntff_profile_hook = lambda h: None
        sys.modules["antenv.axon_hooks"] = mod
        antenv.axon_hooks = mod
        return True
    except Exception as e:  # pragma: no cover
        print(f"ntff hook install failed: {e}", file=sys.stderr)
        return False


def run_on_cores(in_maps, trace=False, trace_all_cores=False):
    """Compile once, run on cores 0..7; optional NTFF profiling."""
    import concourse.bass_utils as bass_utils

    nc = _get_nc()
    tmpdir = None
    trace_cores = None
    if trace:
        trace = _ensure_ntff_hook()
    if trace:
        import tempfile
        tmpdir = tempfile.mkdtemp(prefix="mhsa_ntff_")
        _cache["trace_dir"] = tmpdir
        # no cloud creds in this container; keep artifacts local
        bass_utils.upload_artifacts = lambda d: f"local://{d}"
        if trace_all_cores:
            trace_cores = list(range(N_CORES))
    res = bass_utils.run_bass_kernel_spmd(
        nc, in_maps, list(range(N_CORES)), trace=trace, tmpdir=tmpdir,
        trace_cores=trace_cores)
    _cache["last_results"] = res
    return res


def kernel(x, W_Q, W_K, W_V, W_O):
    x = np.asarray(x, dtype=np.float32)
    in_maps = make_in_maps(x, np.asarray(W_Q, np.float32),
                           np.asarray(W_K, np.float32),
                           np.asarray(W_V, np.float32),
                           np.asarray(W_O, np.float32))
    trace = bool(int(os.environ.get("MHSA_TRACE", "0")))
    all_cores = bool(int(os.environ.get("MHSA_TRACE_ALL_CORES", "0")))
    res = run_on_cores(in_maps, trace=trace, trace_all_cores=all_cores)
    out = np.zeros((NT, D), dtype=np.float32)
    for r in res.results:
        out += np.asarray(r["out"], dtype=np.float32)
    return out.reshape(B, T, D)


# revision 7
# speedup vs baseline: 205.4151x; 1.3220x over previous
"""Multi-head self-attention (B=4, T=2048, D=1024, H=16) on 8 TRN2 NeuronCores.

Sharding: tensor-parallel over heads. Core c owns heads (2c, 2c+1):
  - W_Q/W_K/W_V rows [128c, 128c+128) -> per-core q/k/v of shape [T*B, 128]
  - causal attention for its 2 heads
  - partial output projection through W_O columns [128c, 128c+128)
Host sums the 8 partial outputs (the row-parallel W_O reduction).

Per-core schedule (batch-fused so engines stay dense):
  for b in 0..3:
    phase1(b): q/k/v projections for batch b, one 512-token chunk at a
      time (x streamed chunk-major, ONE dma per chunk).
    attn(b), per 512-q chunk: per 128-k-tile: h-packed score MMs (heads
      at array rows 0-63/64-127, concurrent) -> diag-mask add (DVE) ->
      exp (ACT, bf16 out) -> AV MMs accumulating [64 v-dims | ones] so
      row 64 is the softmax denominator. Diagonal k-tiles are column-
      sliced so fully-masked blocks are never computed.
      Chunk tail: 1/denom via DVE reciprocal_approx_fast (no ACT table
      swaps), GpSimd partition_broadcast, DVE mul into hoT, then the
      output projection + DMA for that chunk's 4 token-tiles.
  The dataflow scheduler overlaps attn(b) with phase1(b+1), keeping the
  PE dense (HAM stays at K=8/8) while ACT runs the exp stream.
"""

import os
import sys

import numpy as np

if "/opt/trn_rl_repo" not in sys.path:
    sys.path.insert(0, "/opt/trn_rl_repo")

import ml_dtypes

B, T, D, NH, DH = 4, 2048, 1024, 16, 64
NT = B * T          # 8192 tokens
MT = D // 128       # 8 model-dim tiles
NCH = NT // 512     # 16 token chunks
N_CORES = 8

_cache = {}


def _build_nc():
    from contextlib import ExitStack

    import concourse.mybir as mybir
    import concourse.tile as tile
    from concourse import bacc

    BF = mybir.dt.bfloat16
    F32 = mybir.dt.float32
    EXP = mybir.ActivationFunctionType.Exp
    LN = mybir.ActivationFunctionType.Ln

    nc = bacc.Bacc("TRN2", target_bir_lowering=False, debug=False)

    # x chunk-major: [chunk, partition, mt, col]
    xT_d = nc.dram_tensor("xT", [NCH, 128, MT, 512], BF, kind="ExternalInput")
    wq_d = nc.dram_tensor("wqT", [MT, 128, 128], BF, kind="ExternalInput")
    wk_d = nc.dram_tensor("wkT", [MT, 128, 128], BF, kind="ExternalInput")
    wv_d = nc.dram_tensor("wvT", [MT, 128, 128], BF, kind="ExternalInput")
    wo_d = nc.dram_tensor("woT", [128, D], BF, kind="ExternalInput")
    cm_d = nc.dram_tensor("cmask", [128, 2, 128], F32, kind="ExternalInput")
    out_d = nc.dram_tensor("out", [NT, D], F32, kind="ExternalOutput")

    with tile.TileContext(nc) as tc, ExitStack() as ctx:
        pers = ctx.enter_context(tc.tile_pool(name="pers", bufs=1))
        wq = pers.tile([128, MT, 128], BF)
        wk = pers.tile([128, MT, 128], BF)
        wv = pers.tile([128, MT, 128], BF)
        wo = pers.tile([128, D], BF)
        cmask = pers.tile([128, 2, 128], F32)

        with tc.tile_pool(name="xc", bufs=3) as xpool, \
             tc.tile_pool(name="qt", bufs=2) as qtp, \
             tc.tile_pool(name="kt", bufs=2) as ktp, \
             tc.tile_pool(name="vbp", bufs=2) as vbp, \
             tc.tile_pool(name="hop", bufs=2) as hop, \
             tc.tile_pool(name="exp", bufs=4) as exp_pool, \
             tc.tile_pool(name="uhp", bufs=2) as uhp, \
             tc.tile_pool(name="lnp", bufs=2) as lnp, \
             tc.tile_pool(name="inv", bufs=2) as invp, \
             tc.tile_pool(name="osb", bufs=3) as osbp, \
             tc.tile_pool(name="p1", bufs=1, space="PSUM") as p1p, \
             tc.tile_pool(name="pop", bufs=1, space="PSUM") as popp, \
             tc.tile_pool(name="sp", bufs=2, space="PSUM") as spp, \
             tc.tile_pool(name="avp", bufs=1, space="PSUM") as avp:

            first = True
            for b in range(B):
                # ---- phase 1: q/k/v projections for batch b ----
                qt = qtp.tile([128, T], BF, tag="qt", name="qt")
                kt = ktp.tile([128, T], BF, tag="kt", name="kt")
                vb = vbp.tile([128, 16, 130], BF, tag="vb", name="vb")
                for lc in range(4):
                    c = 4 * b + lc
                    cs = slice(lc * 512, (lc + 1) * 512)
                    xc = xpool.tile([128, MT, 512], BF, tag="xc", name="xc")
                    nc.sync.dma_start(out=xc, in_=xT_d[c])
                    if first:
                        # weights after the first x chunk so PE starts asap
                        for mt in range(MT):
                            nc.sync.dma_start(out=wq[:, mt, :], in_=wq_d[mt])
                            nc.sync.dma_start(out=wk[:, mt, :], in_=wk_d[mt])
                            nc.sync.dma_start(out=wv[:, mt, :], in_=wv_d[mt])
                        nc.sync.dma_start(out=wo[:], in_=wo_d[:])
                        nc.sync.dma_start(out=cmask, in_=cm_d[:])
                        first = False
                    pq = p1p.tile([128, 512], F32, tag="p1", name="pq")
                    for mt in range(MT):
                        nc.tensor.matmul(pq, wq[:, mt, :], xc[:, mt, :],
                                         start=(mt == 0), stop=(mt == MT - 1))
                    nc.vector.tensor_copy(out=qt[:, cs], in_=pq)
                    pk = p1p.tile([128, 512], F32, tag="p1", name="pk")
                    for mt in range(MT):
                        nc.tensor.matmul(pk, wk[:, mt, :], xc[:, mt, :],
                                         start=(mt == 0), stop=(mt == MT - 1))
                    nc.vector.tensor_copy(out=kt[:, cs], in_=pk)
                    pv = p1p.tile([128, 4, 128], F32, tag="p1", name="pv")
                    for tt in range(4):
                        for mt in range(MT):
                            nc.tensor.matmul(pv[:, tt, :],
                                             xc[:, mt, tt * 128:(tt + 1) * 128],
                                             wv[:, mt, :],
                                             start=(mt == 0), stop=(mt == MT - 1))
                    for h in range(2):
                        nc.vector.tensor_copy(
                            out=vb[:, lc * 4:(lc + 1) * 4, 65 * h:65 * h + 64],
                            in_=pv[:, :, 64 * h:64 * h + 64])
                # softmax-denominator ones columns
                nc.vector.memset(vb[:, :, 64:65], 1.0)
                nc.vector.memset(vb[:, :, 129:130], 1.0)

                # ---- phase 2: causal attention + projection for batch b ----
                hoT = hop.tile([128, T], BF, tag="hoT", name="hoT")
                for qc in range(4):
                    q0 = qc * 512
                    nk = 4 * qc + 4
                    pavs = [avp.tile([65, 512], F32, tag=f"pav{h}",
                                     name=f"pav{h}") for h in range(2)]
                    for kt_i in range(nk):
                        off = 128 * (kt_i - 4 * qc) if kt_i >= 4 * qc else 0
                        pss = spp.tile([128, 2, 512], F32, tag="pss", name="pss")
                        for h in range(2):
                            hp = 64 * h
                            nc.tensor.matmul(
                                pss[:, h, off:512],
                                kt[hp:hp + 64, kt_i * 128:(kt_i + 1) * 128],
                                qt[hp:hp + 64, q0 + off:q0 + 512],
                                start=True, stop=True)
                        if kt_i >= 4 * qc:
                            nc.vector.tensor_add(pss[:, :, off:off + 128],
                                                 pss[:, :, off:off + 128],
                                                 cmask)
                        ex = exp_pool.tile([128, 2, 512], BF, tag="ex", name="ex")
                        nc.scalar.activation(out=ex[:, :, off:512],
                                             in_=pss[:, :, off:512],
                                             func=EXP, scale=0.125)
                        for h in range(2):
                            nc.tensor.matmul(
                                pavs[h][:, off:512],
                                vb[:, kt_i, 65 * h:65 * h + 65],
                                ex[:, h, off:512],
                                start=(kt_i == 0), stop=(kt_i == nk - 1))
                    # chunk tail: stash unnormalized AV+denoms in SBUF
                    # (frees PSUM), 1/denom = exp(-ln d) on ACT (same
                    # table set as the score exps), broadcast, normalize.
                    uh = uhp.tile([128, 2, 512], BF, tag="uh", name="uh")
                    for h in range(2):
                        nc.vector.tensor_copy(out=uh[0:65, h, :],
                                              in_=pavs[h])
                    lnd = lnp.tile([1, 2, 512], F32, tag="lnd", name="lnd")
                    nc.scalar.activation(out=lnd, in_=uh[64:65, :, :],
                                         func=LN)
                    inv = lnp.tile([1, 2, 512], BF, tag="inv", name="inv")
                    with nc.allow_low_precision(
                            reason="softmax 1/denom via exp(-ln d)"):
                        nc.scalar.activation(out=inv, in_=lnd, func=EXP,
                                             scale=-1.0)
                    for h in range(2):
                        invb = invp.tile([64, 512], BF, tag=f"invb{h}",
                                         name=f"invb{h}")
                        nc.gpsimd.partition_broadcast(invb, inv[0:1, h, :])
                        nc.vector.tensor_mul(hoT[64 * h:64 * h + 64,
                                                 q0:q0 + 512],
                                             uh[0:64, h, :], invb)
                    # output projection for this chunk's 4 token-tiles
                    for tt in range(4 * qc, 4 * qc + 4):
                        osb = osbp.tile([128, D], F32, tag="osb", name="osb")
                        for oc in range(2):
                            po = popp.tile([128, 512], F32, tag="po", name="po")
                            nc.tensor.matmul(po,
                                             hoT[:, tt * 128:(tt + 1) * 128],
                                             wo[:, oc * 512:(oc + 1) * 512],
                                             start=True, stop=True)
                            nc.vector.tensor_copy(
                                out=osb[:, oc * 512:(oc + 1) * 512], in_=po)
                        to = b * T + tt * 128
                        nc.sync.dma_start(out=out_d[to:to + 128, :], in_=osb)
    # Compile with the activation-table chooser steered to the one set
    # that contains BOTH exp and ln, so the kernel needs a single
    # ACT_TABLE_LOAD (the default per-function choice alternates between
    # exp_and_others and natural_log, reloading tables at every switch).
    # Set ids stay valid: the dict keeps its size and insertion order,
    # only the membership used for selection is narrowed.
    import concourse.bacc as bacc_mod
    orig_tables = bacc_mod.get_activation_tables

    def _steered_tables(arch):
        tabs = orig_tables(arch)
        keep = "natural_log_exp_and_others"
        if keep in tabs:
            tabs = {name: (fns if name == keep else fns - tabs[keep])
                    for name, fns in tabs.items()}
        return tabs

    bacc_mod.get_activation_tables = _steered_tables
    try:
        nc.compile()
    finally:
        bacc_mod.get_activation_tables = orig_tables
    return nc


def _get_nc():
    if "nc" not in _cache:
        _cache["nc"] = _build_nc()
    return _cache["nc"]


def _bf(a):
    return np.ascontiguousarray(a, dtype=np.float32).astype(ml_dtypes.bfloat16)


def make_in_maps(x, W_Q, W_K, W_V, W_O):
    xT = _bf(x.reshape(NT, D).T)                      # [D, NT]
    # [chunk, partition, mt, col]
    xTc = np.ascontiguousarray(
        xT.reshape(MT, 128, NCH, 512).transpose(2, 1, 0, 3))
    cmask = np.zeros((128, 2, 128), dtype=np.float32)
    for kp in range(128):
        cmask[kp, :, :kp] = -1e10
    in_maps = []
    for c in range(N_CORES):
        rs = slice(c * 128, (c + 1) * 128)
        in_maps.append({
            "xT": xTc,
            "wqT": _bf(W_Q[rs, :].T).reshape(MT, 128, 128),
            "wkT": _bf(W_K[rs, :].T).reshape(MT, 128, 128),
            "wvT": _bf(W_V[rs, :].T).reshape(MT, 128, 128),
            "woT": _bf(W_O[:, rs].T),
            "cmask": cmask,
        })
    return in_maps


def _ensure_ntff_hook():
    """Install antenv.axon_hooks shim (missing in this image) so
    run_bass_kernel_spmd(trace=True) can capture NTFF profiles."""
    try:
        from antenv import axon_hooks  # noqa: F401
        return True
    except ImportError:
        pass
    try:
        import contextlib
        import ctypes
        import types

        import antenv

        so_path = "/opt/axon/libaxon_pjrt.so"
        lib = ctypes.CDLL(so_path)
        if not hasattr(lib, "axon_start_nrt_profile"):
            return False
        lib.axon_start_nrt_profile.argtypes = [
            ctypes.POINTER(ctypes.c_int64), ctypes.c_size_t]
        lib.axon_start_nrt_profile.restype = ctypes.c_int64
        lib.axon_stop_nrt_profile.argtypes = [ctypes.c_char_p]
        lib.axon_stop_nrt_profile.restype = ctypes.c_int64

        @contextlib.contextmanager
        def _hook(output_dir, device_ids):
            import jax
            jax.devices()
            if device_ids:
                ids = (ctypes.c_int64 * len(device_ids))(*device_ids)
                rc = lib.axon_start_nrt_profile(ids, len(device_ids))
            else:
                rc = lib.axon_start_nrt_profile(None, 0)
            if rc != 0:
                raise RuntimeError(f"axon_start_nrt_profile rc={rc}")
            try:
                yield
            finally:
                n = lib.axon_stop_nrt_profile(str(output_dir).encode())
                print(f"ntff profile: {n} file(s) -> {output_dir}",
                      file=sys.stderr)

        mod = types.ModuleType("antenv.axon_hooks")
        mod.get_axon_ntff_profile_hook = lambda: _hook
        mod.set_axon_ntff_profile_hook = lambda h: None
        sys.modules["antenv.axon_hooks"] = mod
        antenv.axon_hooks = mod
        return True
    except Exception as e:  # pragma: no cover
        print(f"ntff hook install failed: {e}", file=sys.stderr)
        return False


def run_on_cores(in_maps, trace=False, trace_all_cores=False):
    """Compile once, run on cores 0..7; optional NTFF profiling."""
    import concourse.bass_utils as bass_utils

    nc = _get_nc()
    tmpdir = None
    trace_cores = None
    if trace:
        trace = _ensure_ntff_hook()
    if trace:
        import tempfile
        tmpdir = tempfile.mkdtemp(prefix="mhsa_ntff_")
        _cache["trace_dir"] = tmpdir
        # no cloud creds in this container; keep artifacts local
        bass_utils.upload_artifacts = lambda d: f"local://{d}"
        if trace_all_cores:
            trace_cores = list(range(N_CORES))
    res = bass_utils.run_bass_kernel_spmd(
        nc, in_maps, list(range(N_CORES)), trace=trace, tmpdir=tmpdir,
        trace_cores=trace_cores)
    _cache["last_results"] = res
    return res


def kernel(x, W_Q, W_K, W_V, W_O):
    x = np.asarray(x, dtype=np.float32)
    in_maps = make_in_maps(x, np.asarray(W_Q, np.float32),
                           np.asarray(W_K, np.float32),
                           np.asarray(W_V, np.float32),
                           np.asarray(W_O, np.float32))
    trace = bool(int(os.environ.get("MHSA_TRACE", "0")))
    all_cores = bool(int(os.environ.get("MHSA_TRACE_ALL_CORES", "0")))
    res = run_on_cores(in_maps, trace=trace, trace_all_cores=all_cores)
    out = np.zeros((NT, D), dtype=np.float32)
    for r in res.results:
        out += np.asarray(r["out"], dtype=np.float32)
    return out.reshape(B, T, D)
